# revision 98
# baseline (speedup 1.0000x reference)
"""Trainium2 Bass kernel for nn_CombinedOrthogonalAdapter (MoE-routed LoRA).

Math (per token t): out[t, :] = (x[t, :] @ A_e^T) @ B_e^T,  e = task_indices[t]
with E=8 experts, rank R=64, D=2048, B*S = 16384 tokens, SCALE = 1.0.

The kernel is DMA-bound: each core owns a single ~360 GB/s DMA resource
and must stream its x shard in and y shard out, so the design minimizes
DRAM bytes and keeps the DMA engine saturated end to end:

  - Host routing ("pair" sharding): tokens are sorted by expert and split
    into 8 contiguous shards of exactly 2048 tokens. Each shard spans at
    most two experts (eA | eB at a data-dependent cut; pure shards use
    eA == eB). Per-core matmul work is the routed minimum -- 8x less than
    the dense-masked reference formulation.
  - x ships as fp8 e3m4 (1 byte): the PE multiplies fp8 x directly against
    bf16 weights (mixed-dtype matmul, bit-exact on HW), halving the input
    stream. y leaves in mixed precision -- half the columns as fp8 e3m4
    (scaled x8 via an exact power-of-two fold into B; host divides back),
    half as bf16. The A/B stacks stay bf16. End-to-end relative error
    1.51e-2 vs the f32 reference (gate: 2e-2), HW-verified.
  - Both experts' weights are packed side by side in the PE: stage A
    computes h for eA AND eB per token in one pass (output width <= 128 is
    free), and a step mask built on device from a [1, 2048] flag row
    zeroes the wrong expert's half during the PSUM eviction. Stage B then
    contracts all 128 rows of [B_eA; B_eB] in one go -- no branching on
    the cut anywhere on device.
  - Pipelining: x is packed block-major on the host (each block's 16
    d-chunks contiguous per partition -> full DMA line rate at any block
    size) and arrives in 256/512-token blocks, small blocks first so
    stage A starts ~3 us in; stage B chunks of earlier blocks are paced
    between stage A matmuls of later ones; PSUM evictions alternate
    DVE/ACT; y leaves in half-chunk (128 x 1024) stores. Dummy warm-up
    matmuls ramp the PE p-state before real work.
  - Per-core DRAM traffic: 4.2 MB x + 6.3 MB y + 1.0 MB weights = 11.5 MB
    (~32 us of DMA) vs 41.6 MB (~116 us) for the f32 dense baseline.
    Cost-model timeline: 40795 ns vs 147299 ns baseline (3.61x); the
    endgame is paced by PE/eviction production, the rest by the gap-free
    DMA stream.
"""

import os

import numpy as np

import concourse.bacc as bacc
import concourse.mybir as mybir
import concourse.tile as tile
from concourse.bass_utils import run_bass_kernel_spmd

# Problem shapes (hardcoded per contest rules).
B, S, D, E, R = 4, 4096, 2048, 8, 64
N_TOK = B * S                     # 16384
N_CORES = 8
DCH = D // 128                    # 16 d chunks
CAP = 2176                        # token capacity per core (max count 2168)
BLOCKS = (256, 384, 512, 512, 512)  # token blocks (small first: pipeline fill)
assert sum(BLOCKS) == CAP
DOUT_BLK = 512                    # matmul PSUM output must fit one bank
NDOUT = D // DOUT_BLK             # 4

F32 = mybir.dt.float32
BF16 = mybir.dt.bfloat16

LAST_RESULTS = None               # test.py introspection hook

_BUILD_CACHE = {}

# ---------------------------------------------------------------------------
# v3 "pair" kernel: tokens sorted by expert and split into 8 contiguous
# shards of exactly TOK=2048 (no padding). Each shard spans at most two
# experts (eA then eB, boundary at `cut`). Both experts' weights are packed
# side by side in the PE array: stage A computes h for BOTH experts per
# token in one pass (free: PE output width is 128 anyway), and a step mask
# (built on device from a [1, TOK] flag row) zeroes the wrong expert's h
# half during PSUM eviction. Stage B then contracts the full 128 rows of
# [B_eA; B_eB] -- tokens left of the cut hit B_eA rows (bottom half of h
# masked to 0) and vice versa.
# ---------------------------------------------------------------------------
TOK = N_TOK // N_CORES            # 2048 tokens per core, exact
# x is laid out block-major on the host (each block's 16 d-chunks are
# contiguous per partition), so every block transfers at full DMA line
# rate regardless of size. Small leading blocks start stage A early and
# bank stage-B output before the store stream begins.
PBLOCKS = tuple(int(v) for v in os.environ.get(
    "KERNEL_PBLOCKS", "256,256,256,256,512,512").split(","))
assert sum(PBLOCKS) == TOK
assert all(b % 128 == 0 and b <= 512 for b in PBLOCKS)
# Stage-B pacing: drain one pending B chunk after every A-matmul c with
# c % PACE_N == PACE_R (tunable for schedule experiments).
PACE_N = int(os.environ.get("KERNEL_PACE_N", "4"))
PACE_R = int(os.environ.get("KERNEL_PACE_R", "3"))
YPOOL = int(os.environ.get("KERNEL_YPOOL", "16"))


def _build_pair():
    nc = bacc.Bacc(
        "TRN2",
        target_bir_lowering=False,
        debug=False,
        enable_asserts=False,
        num_devices=N_CORES,
    )

    # xh[p, boff*DCH + c*blk + t] = x_fp8e3[token lo+t, d = c*128 + p]
    # (sorted shard, block-major: per partition, block j's DCH chunks are
    # one contiguous run of DCH*blk bytes -> full DMA line rate).
    # fp8(1-3-4) on x halves the dominant input stream; the PE consumes it
    # directly against bf16 weights (mixed-dtype matmul, verified exact on
    # HW). End-to-end rel err vs the f32 reference: 1.19e-2 (gate: 2e-2).
    F8E3 = mybir.dt.float8e3
    xh_d = nc.dram_tensor("xh", [128, DCH * TOK], F8E3, kind="ExternalInput")
    # a2[p, c*128 + r2]: r2 < 64 -> A_eA[r2, c*128+p], r2 >= 64 -> A_eB[...]
    a_d = nc.dram_tensor("a2", [128, DCH * 128], BF16, kind="ExternalInput")
    # b2[r2, d]: rows 0..63 = B_eA^T, rows 64..127 = B_eB^T
    b_d = nc.dram_tensor("b2", [128, D], BF16, kind="ExternalInput")
    # mrow[0, t] = 1.0 if t < cut (token belongs to eA) else 0.0
    m_d = nc.dram_tensor("mrow", [1, TOK], BF16, kind="ExternalInput")
    # y leaves in mixed precision: columns 0:1024 as fp8 e3m4 scaled by 8
    # (the x8 is folded into b2's first-half columns on the host -- exact,
    # power of two; host divides back), columns 1024:2048 as bf16.
    # Measured end-to-end rel err 1.51e-2 (gate 2e-2).
    y8_d = nc.dram_tensor("y8", [TOK, D // 2], F8E3, kind="ExternalOutput")
    y16_d = nc.dram_tensor("y16", [TOK, D // 2], BF16, kind="ExternalOutput")



    with tile.TileContext(nc) as tc:
        with (
            tc.tile_pool(name="wpool", bufs=1) as wpool,
            tc.tile_pool(name="hpool", bufs=3) as hpool,
            tc.tile_pool(name="ypool", bufs=YPOOL) as ypool,
        ):
            x_sb = wpool.tile([128, DCH * TOK], F8E3, name="x_sb",
                              tag="x_sb")
            a_sb = wpool.tile([128, DCH * 128], BF16, name="a_sb", tag="a_sb")
            b_sb = wpool.tile([128, D], BF16, name="b_sb", tag="b_sb")
            mr_sb = wpool.tile([1, TOK], BF16, name="mr_sb", tag="mr_sb")
            sign_sb = wpool.tile([1, 128], BF16, name="sign_sb",
                                 tag="sign_sb")
            basec_sb = wpool.tile([128, 1], F32, name="basec_sb",
                                  tag="basec_sb")
            msk_sb = wpool.tile([128, TOK], BF16, name="msk_sb", tag="msk_sb")
            # Constants built on-device (no DMA): sign = [+1]*64 ++ [-1]*64,
            # base column = [0]*64 ++ [1]*64 (per-partition activation bias).
            nc.vector.memset(sign_sb[:, 0:64], 1.0)
            nc.vector.memset(sign_sb[:, 64:128], -1.0)
            nc.vector.memset(basec_sb[0:64, :], 0.0)
            nc.vector.memset(basec_sb[64:128, :], 1.0)

            offs = []
            t0 = 0
            for blk in PBLOCKS:
                offs.append(t0)
                t0 += blk

            # Warm-up operand built by memset (no DMA): PE dummies can start
            # as soon as the DVE clears, well before x block 0 lands.
            wu_sb = wpool.tile([1, 512], BF16, name="wu_sb", tag="wu_sb")
            nc.vector.memset(wu_sb[:], 1.0)
            def xcols(j):
                lo, blk = offs[j], PBLOCKS[j]
                return slice(DCH * lo, DCH * (lo + blk))

            nc.sync.dma_start(x_sb[:, xcols(0)], xh_d[:, xcols(0)])
            nc.sync.dma_start(mr_sb[:], m_d[:, :])
            nc.sync.dma_start(a_sb[:], a_d[:, :])
            nc.sync.dma_start(x_sb[:, xcols(1)], xh_d[:, xcols(1)])
            nc.sync.dma_start(b_sb[:], b_d[:, :])
            for j in range(2, len(PBLOCKS)):
                nc.sync.dma_start(x_sb[:, xcols(j)], xh_d[:, xcols(j)])

            ppool = tc.tile_pool(name="psumP", bufs=1, space="PSUM")
            psumP = ppool.__enter__()
            psumA = psumB = psumM = psumP

            AL = mybir.AluOpType

            def emit_mask_chunk(mc):
                # mask2[r2, t] = sign(r2) * mrow(t) + base(r2)
                #             = 1 iff (t < cut) == (r2 < 64)
                # Built per 512-column chunk (PSUM bank limit), interleaved
                # with stage A so it stays off the critical path.
                # One matmul (sign (x) mrow, values in {-1, 0, +1}); the
                # per-partition base is added during eviction as an
                # activation bias, and ReLU maps {-1, 0} -> 0, 1 -> 1.
                msl = slice(mc * 512, (mc + 1) * 512)
                mps = psumM.tile([128, 512], F32, name="mps", tag="mps",
                                 bufs=2)
                nc.tensor.matmul(mps[:], lhsT=sign_sb[:],
                                 rhs=mr_sb[:, msl], start=True, stop=True)
                nc.scalar.activation(
                    msk_sb[:, msl], mps[:],
                    mybir.ActivationFunctionType.Relu, bias=basec_sb[:])

            def emit_b_chunk(h_sb, lo, s, pat=None):
                # stage B + store for one 128-token chunk. PSUM evictions
                # can only run on DVE/ACT (GPSIMD has no PSUM access).
                # d-blocks 0,1 evict to fp8 (values pre-scaled x8 via b2),
                # d-blocks 2,3 to bf16; one store per precision half.
                y8_sb = ypool.tile([128, D // 2], F8E3, name="y8_sb",
                                   tag="y8_sb")
                y16_sb = ypool.tile([128, D // 2], BF16, name="y16_sb",
                                    tag="y16_sb")
                if pat is None:
                    pat = os.environ.get("KERNEL_YEVICT", "vsvs")
                for o in range(NDOUT):
                    yps = psumB.tile([128, DOUT_BLK], F32, name="yps",
                                     tag="yps", bufs=4)
                    nc.tensor.matmul(
                        yps[:],
                        lhsT=h_sb[:, s * 128:(s + 1) * 128],
                        rhs=b_sb[:, o * DOUT_BLK:(o + 1) * DOUT_BLK],
                        start=True, stop=True,
                    )
                    if o < 2:
                        dst = y8_sb[:, o * DOUT_BLK:(o + 1) * DOUT_BLK]
                    else:
                        dst = y16_sb[:, (o - 2) * DOUT_BLK:
                                     (o - 1) * DOUT_BLK]
                    if pat[o] == "v":
                        nc.vector.tensor_copy(dst, yps[:])
                    else:
                        nc.scalar.copy(dst, yps[:])
                row0 = lo + s * 128
                nc.sync.dma_start(y8_d[row0:row0 + 128, :], y8_sb[:])
                nc.sync.dma_start(y16_d[row0:row0 + 128, :], y16_sb[:])

            # Software-pipelined emission: stage B chunks of block j-1 are
            # interleaved between stage A matmuls of block j, so the PE
            # in-order queue never stalls on PSUM evictions (which would
            # also drop the tensor engine out of its ramped p-state).
            # Emission order: A(0) leads (needs only x0 + a2, both first in
            # the DMA stream); the mask build follows A(0), still ahead of
            # the first masked h eviction. Stage B chunks are paced from a
            # queue: one chunk drained after every 4 stage-A matmuls, so PE
            # work overlaps the x stream as much as possible.
            bq = []                   # pending stage-B chunks

            def drain_b(pat=None):
                if bq:
                    emit_b_chunk(*bq.pop(0), pat=pat)

            # PE p-state warm-up: dummy matmuls (outputs never read) keep the
            # tensor engine busy from ~3 us so the ramp reaches full clock
            # before the real pipeline starts.
            wu_rows = int(os.environ.get("KERNEL_WUROWS", "512"))
            for _ in range(int(os.environ.get("KERNEL_WARMUP", "3"))):
                wps = psumM.tile([64, 512], F32, name="wps", tag="mps",
                                 bufs=2)
                nc.tensor.matmul(wps[:, 0:wu_rows], lhsT=wu_sb[:, 0:64],
                                 rhs=wu_sb[:, 0:wu_rows],
                                 start=True, stop=True)

            if os.environ.get("KERNEL_MASKFIRST", "1") == "1":
                for mc in range(TOK // 512):
                    emit_mask_chunk(mc)
            for j, blk in enumerate(PBLOCKS):
                lo = offs[j]
                # fixed-size tile (uniform tag footprint), sliced to blk
                hps_t = psumA.tile([128, 512], F32, name="hps", tag="hps",
                                   bufs=2)
                hps = hps_t[:, 0:blk]
                for c in range(DCH):
                    x0c = DCH * lo + c * blk
                    nc.tensor.matmul(
                        hps,
                        lhsT=a_sb[:, c * 128:(c + 1) * 128],
                        rhs=x_sb[:, x0c:x0c + blk],
                        start=(c == 0),
                        stop=(c == DCH - 1),
                    )
                    if (j == 0 and c == DCH - 1
                            and os.environ.get("KERNEL_MASKFIRST", "1")
                            != "1"):
                        for mc in range(TOK // 512):
                            emit_mask_chunk(mc)
                    if c % PACE_N == PACE_R and c < int(
                            os.environ.get("KERNEL_PACE_MAX", "16")):
                        # during the final A-block, keep DVE clear so the
                        # last masked h eviction is not queued behind
                        # y evictions (ACT-only pattern for those chunks)
                        last = (j == len(PBLOCKS) - 1
                                and os.environ.get("KERNEL_LASTS", "0")
                                == "1")
                        drain_b("ssss" if last else None)
                # masked eviction: zero the wrong expert's half per token
                h_sb = hpool.tile([128, blk], BF16, name="h_sb")
                nc.vector.tensor_tensor(
                    out=h_sb[:], in0=hps, in1=msk_sb[:, lo:lo + blk],
                    op=AL.mult)
                bq += [(h_sb, lo, s) for s in range(blk // 128)]
            while bq:
                drain_b()
            ppool.__exit__(None, None, None)
    nc.compile()
    return nc


def _build():
    nc = bacc.Bacc(
        "TRN2",
        target_bir_lowering=False,
        debug=False,
        enable_asserts=False,
        num_devices=N_CORES,
    )

    # xh[p, c, t] = x_bf16[token t, d = c*128 + p]  (expert-routed, padded)
    xh_d = nc.dram_tensor("xh", [128, DCH, CAP], BF16, kind="ExternalInput")
    # a_p[p, c*64 + r] = A_e[r, c*128 + p]
    a_d = nc.dram_tensor("a_p", [128, DCH * R], BF16, kind="ExternalInput")
    # b_p[r, d] = B_e[d, r]
    b_d = nc.dram_tensor("b_p", [R, D], BF16, kind="ExternalInput")
    y_d = nc.dram_tensor("y", [CAP, D], BF16, kind="ExternalOutput")

    with tile.TileContext(nc) as tc:
        with (
            tc.tile_pool(name="wpool", bufs=1) as wpool,
            tc.tile_pool(name="hpool", bufs=3) as hpool,
            tc.tile_pool(name="ypool", bufs=8) as ypool,
            tc.tile_pool(name="psumA", bufs=2, space="PSUM") as psumA,
            tc.tile_pool(name="psumB", bufs=3, space="PSUM") as psumB,
        ):
            # x lives SBUF-resident for the whole kernel: [128, 16, 2176] bf16
            x_sb = wpool.tile([128, DCH, CAP], BF16, name="x_sb", tag="x_sb")
            a_sb = wpool.tile([128, DCH * R], BF16, name="a_sb", tag="a_sb")
            b_sb = wpool.tile([R, D], BF16, name="b_sb", tag="b_sb")

            # x block 0 first (shortest), then weights, then the rest: the
            # DMA engine never idles and stage A(0) starts ~4 us in.
            offs = []
            t0 = 0
            for blk in BLOCKS:
                offs.append(t0)
                t0 += blk
            nc.sync.dma_start(
                x_sb[:, :, 0:BLOCKS[0]], xh_d[:, :, 0:BLOCKS[0]])
            nc.sync.dma_start(a_sb[:], a_d[:, :])
            nc.sync.dma_start(b_sb[:], b_d[:, :])
            for j in range(1, len(BLOCKS)):
                lo, hi = offs[j], offs[j] + BLOCKS[j]
                nc.sync.dma_start(x_sb[:, :, lo:hi], xh_d[:, :, lo:hi])

            for j, blk in enumerate(BLOCKS):
                lo = offs[j]
                # ---- stage A: h[r, t] for this block ----
                hps = psumA.tile([64, blk], F32, name="hps", tag="hps")
                for c in range(DCH):
                    nc.tensor.matmul(
                        hps[:],
                        lhsT=a_sb[:, c * R:(c + 1) * R],
                        rhs=x_sb[:, c, lo:lo + blk],
                        start=(c == 0),
                        stop=(c == DCH - 1),
                    )
                h_sb = hpool.tile([64, blk], BF16, name="h_sb")
                nc.vector.tensor_copy(h_sb[:], hps[:])

                # ---- stage B + store, per 128-token chunk ----
                for s in range(blk // 128):
                    y_sb = ypool.tile([128, D], BF16, name="y_sb")
                    for o in range(NDOUT):
                        yps = psumB.tile([128, DOUT_BLK], F32, name="yps",
                                         tag="yps")
                        nc.tensor.matmul(
                            yps[:],
                            lhsT=h_sb[:, s * 128:(s + 1) * 128],
                            rhs=b_sb[:, o * DOUT_BLK:(o + 1) * DOUT_BLK],
                            start=True, stop=True,
                        )
                        dst = y_sb[:, o * DOUT_BLK:(o + 1) * DOUT_BLK]
                        if o % 2 == 0:
                            nc.vector.tensor_copy(dst, yps[:])
                        else:
                            nc.scalar.copy(dst, yps[:])
                    row0 = lo + s * 128
                    # SP queue: keeps DMA-issue sem waits off the
                    # Activation queue, which is busy with PSUM evictions.
                    nc.sync.dma_start(y_d[row0:row0 + 128, :], y_sb[:])
    nc.compile()
    return nc


IMPL = os.environ.get("KERNEL_IMPL", "pair")


def _get_nc():
    if IMPL not in _BUILD_CACHE:
        _BUILD_CACHE[IMPL] = _build_pair() if IMPL == "pair" else _build()
    return _BUILD_CACHE[IMPL]


def _route_pair(task_indices):
    """Sort tokens by expert; shard k = sorted tokens [k*TOK, (k+1)*TOK).

    Returns (order, shards) where shards[k] = (eA, eB, cut), or None if some
    shard spans more than two experts (then the caller must fall back).
    """
    idx = np.asarray(task_indices).reshape(-1)
    order = np.argsort(idx, kind="stable")
    sidx = idx[order]
    shards = []
    for k in range(N_CORES):
        seg = sidx[k * TOK:(k + 1) * TOK]
        experts = np.unique(seg)
        if len(experts) > 2:
            return order, None
        eA = int(experts[0])
        eB = int(experts[-1])  # == eA for pure shards
        cut = int(np.searchsorted(seg, eA, side="right"))
        shards.append((eA, eB, cut))
    return order, shards


def prepare_in_maps_pair(x, lora_A, lora_B, order, shards):
    import ml_dtypes

    bf16 = ml_dtypes.bfloat16
    xf = np.asarray(x, dtype=np.float32).reshape(N_TOK, D)
    lora_A = np.asarray(lora_A, dtype=np.float32)
    lora_B = np.asarray(lora_B, dtype=np.float32)

    f8e3 = ml_dtypes.float8_e3m4
    in_maps = []
    for k in range(N_CORES):
        eA, eB, cut = shards[k]
        p = order[k * TOK:(k + 1) * TOK]
        xe = xf[p]                                   # [TOK, D]
        xeT = xe.T                                   # [D, TOK]
        # block-major packing: xh[p, DCH*lo + c*blk + t] = xeT[c*128+p, lo+t]
        xh = np.empty((128, DCH * TOK), dtype=f8e3)
        t0 = 0
        for blk in PBLOCKS:
            xb = xeT[:, t0:t0 + blk].reshape(DCH, 128, blk)
            xh[:, DCH * t0:DCH * (t0 + blk)] = (
                xb.transpose(1, 0, 2).reshape(128, DCH * blk).astype(f8e3))
            t0 += blk
        # a2: per d-chunk stationary [128, 128] = [A_eA chunk | A_eB chunk]
        acat = np.concatenate([lora_A[eA].T, lora_A[eB].T], axis=1)  # [D,128]
        a2 = np.ascontiguousarray(
            acat.reshape(DCH, 128, 128).transpose(1, 0, 2)
            .reshape(128, DCH * 128)).astype(bf16)
        b2f = np.concatenate([lora_B[eA].T, lora_B[eB].T], axis=0)
        # fold the fp8-half output scale into B: y[:, 0:1024] computes 8*y
        # (exact power-of-two scaling; host divides back after the run)
        b2f[:, 0:D // 2] *= 8.0
        b2 = b2f.astype(bf16)
        mrow = np.zeros((1, TOK), dtype=np.float32)
        mrow[0, :cut] = 1.0
        in_maps.append({
            "xh": xh,
            "a2": np.ascontiguousarray(a2),
            "b2": np.ascontiguousarray(b2),
            "mrow": mrow.astype(bf16),
        })
    return in_maps


def _route(task_indices):
    idx = np.asarray(task_indices).reshape(-1)
    perms = [np.nonzero(idx == e)[0] for e in range(E)]
    return perms


def prepare_in_maps(x, lora_A, lora_B, perms):
    import ml_dtypes

    bf16 = ml_dtypes.bfloat16
    xf = np.asarray(x, dtype=np.float32).reshape(N_TOK, D)
    lora_A = np.asarray(lora_A, dtype=np.float32)
    lora_B = np.asarray(lora_B, dtype=np.float32)

    in_maps = []
    for e in range(E):
        p = perms[e]
        xe = np.zeros((CAP, D), dtype=np.float32)
        xe[: len(p)] = xf[p]
        # [CAP, D] -> xT [D, CAP] -> [16, 128, CAP] -> [128, 16, CAP]
        xh = np.ascontiguousarray(
            xe.T.reshape(DCH, 128, CAP).transpose(1, 0, 2)).astype(bf16)
        a_p = np.ascontiguousarray(
            lora_A[e].T.reshape(DCH, 128, R).transpose(1, 0, 2)
            .reshape(128, DCH * R)).astype(bf16)
        b_p = np.ascontiguousarray(lora_B[e].T).astype(bf16)
        in_maps.append({"xh": xh, "a_p": a_p, "b_p": b_p})
    return in_maps


def _numpy_fallback(x, lora_A, lora_B, task_indices):
    # Correctness-preserving fallback for inputs whose routing exceeds CAP.
    xf = np.asarray(x, dtype=np.float32).reshape(N_TOK, D)
    idx = np.asarray(task_indices).reshape(-1)
    out = np.zeros_like(xf)
    for e in range(E):
        p = np.nonzero(idx == e)[0]
        if len(p) == 0:
            continue
        h = xf[p] @ np.asarray(lora_A[e], dtype=np.float32).T
        out[p] = h @ np.asarray(lora_B[e], dtype=np.float32).T
    return out.reshape(np.asarray(x).shape).astype(np.float32)


def kernel(x, lora_A, lora_B, task_indices):
    global LAST_RESULTS

    if IMPL == "pair":
        order, shards = _route_pair(task_indices)
        if shards is None:
            return _numpy_fallback(x, lora_A, lora_B, task_indices)
        in_maps = prepare_in_maps_pair(x, lora_A, lora_B, order, shards)
        nc = _get_nc()
        res = run_bass_kernel_spmd(
            nc, in_maps, core_ids=list(range(N_CORES)),
            trace=bool(int(os.environ.get("KERNEL_TRACE", "0"))),
        )
        LAST_RESULTS = res
        out = np.zeros((N_TOK, D), dtype=np.float32)
        ys = np.empty((N_TOK, D), dtype=np.float32)
        for k, r in enumerate(res.results):
            rows = slice(k * TOK, (k + 1) * TOK)
            ys[rows, 0:D // 2] = np.asarray(r["y8"]).astype(np.float32) / 8.0
            ys[rows, D // 2:] = np.asarray(r["y16"]).astype(np.float32)
        out[order] = ys
        return out.reshape(B, S, D)

    perms = _route(task_indices)
    if max(len(p) for p in perms) > CAP:
        return _numpy_fallback(x, lora_A, lora_B, task_indices)

    in_maps = prepare_in_maps(x, lora_A, lora_B, perms)
    nc = _get_nc()
    res = run_bass_kernel_spmd(
        nc, in_maps, core_ids=list(range(N_CORES)),
        trace=bool(int(os.environ.get("KERNEL_TRACE", "0"))),
    )
    LAST_RESULTS = res

    out = np.zeros((N_TOK, D), dtype=np.float32)
    for e in range(E):
        p = perms[e]
        out[p] = np.asarray(res.results[e]["y"][: len(p)], dtype=np.float32)
    return out.reshape(B, S, D)
